# revision 5
# baseline (speedup 1.0000x reference)
"""Trainium2 Bass kernel for DilatedReparamConv (5-branch depthwise conv + BN + SiLU + identity BN).

out = BN_id(x) + sum_i silu(BN_i(dwconv_i(x)))   for branches
      (5,d1), (7,d2), (3,d3), (3,d4), (3,d5), all SAME padding.

Strategy (8 NeuronCores, SPMD, channel-sharded: 32 ch/core = 16 pairs):
  - Depthwise conv on TensorE in fp8 e4m3 DoubleRowSwInterleave. Per kw
    tap, a banded block-diagonal Toeplitz over (2ch x 64h) contracts the
    kh stack. The two DR slots per stream carry either
      * feedback: (T8, T8/2) x (x8, r)  ->  T8 @ x  (x at ~0.25%), or
      * pure pair: (T8[jA], T8[jB]) x (x8 shifted dxA, x8 shifted dxB)
        -> TWO kw taps in one stream (x at fp8, ~2% RMS).
    STREAM_MODE picks the split: "fb21" = all-feedback (21 streams,
    max accuracy) or "hyb16" = feedback on the heavy 3x3 branches +
    highest-energy tap of br0/br1, pure pairs elsewhere (16 streams,
    ~24% less PE streaming).
  - Per (branch, pair): one [128, 2048] PSUM tile (4 banks) accumulated
    chunk-outer/tap-inner; single ScalarE Silu eviction (BN fused) to
    fp16.
  - Branch sum: fp16 tensor_tensor tree on VectorE (2-byte 2x mode);
    identity = tensor_scalar affine from fp16 natural x + final add.
  - Output fp16; host casts to f32.
"""

import sys

sys.path.insert(0, "/opt/trn_rl_repo")

import numpy as np
import ml_dtypes

import bass_rust
import concourse.bass as bass
import concourse.mybir as mybir
from concourse import bacc, tile
from concourse.bass_utils import run_bass_kernel_spmd

# ---------------------------------------------------------------- problem dims
B, C, H, W = 32, 256, 64, 64
EPS = 1e-5
BRANCH_CFG = [(5, 1), (7, 2), (3, 3), (3, 4), (3, 5)]  # (kernel, dilation)
N_CORES = 8
C_CORE = C // N_CORES          # 32 channels per core
PAIRS = C_CORE // 2            # 16 channel-pairs per core
PAD = 6                        # max dilation*(ks-1)//2 across branches
WP = W + 2 * PAD               # padded width = 76
W_CHUNK, N_CHUNKS = 16, 4      # 512-col psum chunks

STREAM_MODE = "hyb16"          # "fb21" | "hyb16"
# feedback taps per branch: hyb16 = 1,1,all,all,all; fb21 = all
FB_COUNTS = {"fb21": [5, 7, 3, 3, 3], "hyb16": [1, 1, 3, 3, 3]}[STREAM_MODE]

F8 = ml_dtypes.float8_e4m3
F16 = np.float16
BR_ORDER = [1, 0, 2, 3, 4]     # processing order; weights stored in this order

XB = WP * B                    # 2432 fp8 elements per plane


def _branch_taps():
    """per branch: list of (tap_index_in_branch, dx)."""
    out = []
    for br, (ks, dil) in enumerate(BRANCH_CFG):
        pad = dil * (ks - 1) // 2
        out.append([(kw, dil * kw - pad) for kw in range(ks)])
    return out


def _stream_cfg(weights):
    """Per branch, list of streams:
    ('fb', dx, kw) or ('pure', dxA, kwA, dxB, kwB), dxA < dxB."""
    cfg = []
    for br, (ks, dil) in enumerate(BRANCH_CFG):
        taps = _branch_taps()[br]
        e = (np.asarray(weights[br][:, 0], np.float32) ** 2).sum(axis=(0, 1))  # [ks]
        order = np.argsort(e)[::-1]
        n_fb = FB_COUNTS[br]
        fb_kws = set(order[:n_fb].tolist())
        streams = [("fb", dx, kw) for kw, dx in taps if kw in fb_kws]
        pure = sorted([(dx, kw) for kw, dx in taps if kw not in fb_kws])
        assert len(pure) % 2 == 0, f"branch {br}: odd pure taps"
        for i in range(0, len(pure), 2):
            (dxA, kwA), (dxB, kwB) = pure[i], pure[i + 1]
            streams.append(("pure", dxA, kwA, dxB, kwB))
        cfg.append(streams)
    return cfg


def _n_streams(cfg):
    return sum(len(s) for s in cfg)


def _stream_span(st):
    """Valid output w range [lo, hi): outside it every tap reads pad zeros."""
    dxs = [st[1]] if st[0] == "fb" else [st[1], st[3]]
    lo = -max(dxs) if all(d < 0 for d in dxs) else 0
    hi = 64 - min(dxs) if all(d > 0 for d in dxs) else 64
    return lo, hi


def _chunk_order(streams, c0, c1):
    """Issue order for one chunk [c0, c1): full-width first & last (they get
    start/stop), trimmed streams in between. Returns [(stream_idx, a, b)]."""
    full, part = [], []
    for si, st in enumerate(streams):
        lo, hi = _stream_span(st)
        a, b = max(c0, lo), min(c1, hi)
        if a <= c0 and b >= c1:
            full.append((si, c0, c1))
        elif b > a:
            part.append((si, a, b))
    assert len(full) >= 2, f"need 2 full-width streams in chunk [{c0},{c1})"
    return [full[0]] + part + full[1:]


# =====================================================================
# device program
# =====================================================================
def build_nc(cfg):
    nc = bacc.Bacc("TRN2", target_bir_lowering=False, debug=False, num_devices=N_CORES)
    f32 = mybir.dt.float32
    f16 = mybir.dt.float16
    u8 = mybir.dt.uint8
    f8 = mybir.dt.float8e4

    ns = _n_streams(cfg)
    w_off = 2 * XB                     # weights after x8 | r planes
    xn_off = w_off + ns * 256          # fp16 natural x last (non-critical DMA)
    cols = xn_off + 2 * W * B

    xw = nc.dram_tensor("xw", [PAIRS, 128, cols], u8, kind="ExternalInput").ap()
    scbi = nc.dram_tensor("scbi", [128, 2 * PAIRS * 6], f32, kind="ExternalInput").ap()
    yt = nc.dram_tensor("yt", [PAIRS, 128, B * W], f16, kind="ExternalOutput").ap()

    with tile.TileContext(nc) as tc:
        with (
            tc.tile_pool(name="consts", bufs=1) as consts,
            tc.tile_pool(name="xwp", bufs=3) as xwp,
            tc.tile_pool(name="tp", bufs=10) as tp,
            tc.tile_pool(name="up", bufs=6) as up,
            tc.tile_pool(name="accp", bufs=3) as accp,
            tc.tile_pool(name="psum", bufs=2, space="PSUM") as psum,
        ):
            scbi_t = consts.tile([128, 2 * PAIRS * 6], f32)
            nc.sync.dma_start(out=scbi_t[:], in_=scbi)
            sc_t = scbi_t[:, : PAIRS * 6]
            bi_t = scbi_t[:, PAIRS * 6 :]
            # preload the Silu activation table during the first x DMA
            warm = consts.tile([128, 1], f16)
            nc.scalar.activation(
                warm[:], scbi_t[:, 0:1], mybir.ActivationFunctionType.Silu
            )
            # ramp the PE clock while the first DMA is in flight: dummy DR
            # matmuls reading scbi bytes (already loaded; result overwritten)
            jf = scbi_t[:].bitcast(f8)
            jstride = jf.ap[0][0]
            jl = jf[:, 0:256].rearrange("p (r m) -> p r m", r=2)
            jr = jf[:, 0:768].unsqueeze(1).broadcast_to([128, 2, 768]).copy()
            jr.ap = bass_rust.VecI64Pair([(jstride, 128), (256, 2), (1, 512)])

            s_base = {}
            off = 0
            for br in BR_ORDER:
                s_base[br] = off
                off += len(cfg[br])
            w1 = len(cfg[BR_ORDER[0]]) * 256   # leading branch weight bytes
            for p in range(PAIRS):
                xw_t = xwp.tile([128, cols], u8)
                # critical bytes split (x8|r then weights) so matmul data
                # lands earliest; xnat last
                nc.sync.dma_start(out=xw_t[:, :w_off], in_=xw[p][:, :w_off])
                nc.sync.dma_start(
                    out=xw_t[:, w_off : w_off + w1],
                    in_=xw[p][:, w_off : w_off + w1],
                )
                nc.sync.dma_start(
                    out=xw_t[:, w_off + w1 : xn_off],
                    in_=xw[p][:, w_off + w1 : xn_off],
                )
                nc.sync.dma_start(out=xw_t[:, xn_off:], in_=xw[p][:, xn_off:])
                # feedback rhs: [128, 2, XB] planes (x8, r)
                x8r = xw_t[:, : 2 * XB].bitcast(f8).rearrange(
                    "p (r q) -> p r q", r=2
                )
                # pure rhs base: x8 plane alone
                x8f = xw_t[:, :XB].bitcast(f8)
                part_stride = x8f.ap[0][0]
                xnat = xw_t[:, xn_off:].bitcast(f16)
                wt8 = xw_t[:, w_off:xn_off].bitcast(f8).rearrange(
                    "p (s r m) -> p s r m", s=ns, r=2
                )

                def pure_rhs(base, delta, n):
                    sl = x8f[:, base : base + delta + n]
                    ap = sl.unsqueeze(1).broadcast_to([128, 2, delta + n]).copy()
                    ap.ap = bass_rust.VecI64Pair(
                        [(part_stride, 128), (delta, 2), (1, n)]
                    )
                    return ap

                # identity affine early (only needs xnat)
                chain = up.tile([128, B * W], f16)
                nc.vector.tensor_scalar(
                    chain[:],
                    xnat,
                    sc_t[:, p * 6 + 5 : p * 6 + 6],
                    bi_t[:, p * 6 + 5 : p * 6 + 6],
                    mybir.AluOpType.mult,
                    mybir.AluOpType.add,
                )

                for oi, br in enumerate(BR_ORDER):
                    streams = cfg[br]
                    ps = psum.tile([128, N_CHUNKS * W_CHUNK * B], f32)
                    if p == 0 and oi == 0:
                        # ramp the PE clock while the first DMA lands: dummy
                        # DR matmuls on junk; region is overwritten by the
                        # real start=True group below
                        for wi in range(10):
                            nc.tensor.matmul(
                                ps[:, 0:512], jl, jr,
                                start=(wi == 0), stop=(wi == 9),
                                perf_mode=mybir.MatmulPerfMode.DoubleRowSwInterleave,
                            )
                    for cch in range(N_CHUNKS):
                        c0, c1 = cch * W_CHUNK, (cch + 1) * W_CHUNK
                        order = _chunk_order(streams, c0, c1)
                        for k, (si, a, b) in enumerate(order):
                            st = streams[si]
                            n = (b - a) * B
                            if st[0] == "fb":
                                dx = st[1]
                                base = (a + PAD + dx) * B
                                rhs = x8r[:, :, base : base + n]
                            else:
                                _, dxA, _, dxB, _ = st
                                base = (a + PAD + dxA) * B
                                rhs = pure_rhs(base, (dxB - dxA) * B, n)
                            nc.tensor.matmul(
                                ps[:, a * B : b * B],
                                wt8[:, s_base[br] + si],
                                rhs,
                                start=(k == 0),
                                stop=(k == len(order) - 1),
                                perf_mode=mybir.MatmulPerfMode.DoubleRowSwInterleave,
                            )
                    t_br = tp.tile([128, B * W], f16)
                    HB = B * W // 2
                    if oi < 4:
                        nc.scalar.activation(
                            t_br[:],
                            ps[:],
                            mybir.ActivationFunctionType.Silu,
                            bias=bi_t[:, p * 6 + br : p * 6 + br + 1],
                            scale=sc_t[:, p * 6 + br : p * 6 + br + 1],
                        )
                        nxt = up.tile([128, B * W], f16)
                        nc.vector.tensor_tensor(
                            nxt[:], chain[:], t_br[:], op=mybir.AluOpType.add
                        )
                        chain = nxt
                    else:
                        # last branch: halved eviction, add, and out DMA so
                        # the tail pipelines act -> DVE -> DMA
                        acc = accp.tile([128, B * W], f16)
                        for h in range(2):
                            sl = slice(h * HB, (h + 1) * HB)
                            nc.scalar.activation(
                                t_br[:, sl],
                                ps[:, sl],
                                mybir.ActivationFunctionType.Silu,
                                bias=bi_t[:, p * 6 + br : p * 6 + br + 1],
                                scale=sc_t[:, p * 6 + br : p * 6 + br + 1],
                            )
                            nc.vector.tensor_tensor(
                                acc[:, sl],
                                chain[:, sl],
                                t_br[:, sl],
                                op=mybir.AluOpType.add,
                            )
                            nc.sync.dma_start(out=yt[p][:, sl], in_=acc[:, sl])

    nc.compile()
    return nc


# ------------------------------------------------------------------ host prep
def _bn_scale_bias(gamma, beta, mean, var):
    s = gamma / np.sqrt(var + EPS)
    return s, beta - mean * s


def _toeplitz_tap(wcol, dil):
    """T[c, hi, ho] for one kw column's kh stack. wcol [C, ks]."""
    ks = wcol.shape[1]
    pad = dil * (ks - 1) // 2
    T = np.zeros((C, H, H), np.float32)
    for kh in range(ks):
        off = dil * kh - pad
        ho = np.arange(max(0, -off), min(H, H - off))
        T[:, ho + off, ho] = wcol[:, kh][:, None]
    return T


def _host_prep(cfg, x, id_bn, w5, w7, w3a, w3b, w3c, bn_gamma, bn_beta, bn_mean, bn_var):
    x = np.asarray(x, np.float32)
    weights = [np.asarray(w, np.float32) for w in (w5, w7, w3a, w3b, w3c)]
    id_bn = np.asarray(id_bn, np.float32)
    bn_gamma = np.asarray(bn_gamma, np.float32)
    bn_beta = np.asarray(bn_beta, np.float32)
    bn_mean = np.asarray(bn_mean, np.float32)
    bn_var = np.asarray(bn_var, np.float32)

    gx = 8.0 / max(np.abs(x).max(), 1e-8)
    gain = np.empty((5, C), np.float32)
    for br in range(5):
        wmax = np.abs(weights[br][:, 0]).max(axis=(1, 2))
        gain[br] = 8.0 / np.maximum(wmax, 1e-8)

    # BN scale/bias tables; branch scale folds away gain and gx
    S = np.zeros((6, C), np.float32)
    Bv = np.zeros((6, C), np.float32)
    for i in range(5):
        S[i], Bv[i] = _bn_scale_bias(bn_gamma[i], bn_beta[i], bn_mean[i], bn_var[i])
        S[i] = S[i] / (gain[i] * gx)
    S[5], Bv[5] = _bn_scale_bias(id_bn[0], id_bn[1], id_bn[2], id_bn[3])
    scbi = []
    for k in range(N_CORES):
        sck = np.empty((128, PAIRS * 6), np.float32)
        bik = np.empty((128, PAIRS * 6), np.float32)
        for p in range(PAIRS):
            for i in range(6):
                for ci in range(2):
                    c = k * C_CORE + 2 * p + ci
                    sck[ci * H : (ci + 1) * H, p * 6 + i] = S[i, c]
                    bik[ci * H : (ci + 1) * H, p * 6 + i] = Bv[i, c]
        scbi.append(np.ascontiguousarray(np.concatenate([sck, bik], axis=1)))

    # x planes
    xp = np.zeros((C, H, WP, B), np.float32)
    xp[:, :, PAD : PAD + W, :] = np.transpose(x, (1, 2, 3, 0)) * gx
    x8 = xp.astype(F8)
    r8 = (2.0 * (xp - x8.astype(np.float32))).astype(F8)
    xn = np.ascontiguousarray(np.transpose(x, (1, 2, 3, 0)).astype(F16))

    # per-stream slot matrices [C, 2, H, H] fp8
    ns = _n_streams(cfg)
    slotT = np.zeros((C, ns, 2, H, H), F8)
    s_idx = 0
    for br in BR_ORDER:
        ks, dil = BRANCH_CFG[br]
        wbr = weights[br][:, 0] * gain[br][:, None, None]  # [C, ks, ks] scaled
        T8 = {}
        for kw in range(ks):
            T8[kw] = _toeplitz_tap(wbr[:, :, kw], dil).astype(F8)
        for si, st in enumerate(cfg[br]):
            if st[0] == "fb":
                kw = st[2]
                slotT[:, s_idx + si, 0] = T8[kw]
                slotT[:, s_idx + si, 1] = (
                    T8[kw].astype(np.float32) / 2.0
                ).astype(F8)
            else:
                _, _, kwA, _, kwB = st
                slotT[:, s_idx + si, 0] = T8[kwA]
                slotT[:, s_idx + si, 1] = T8[kwB]
        s_idx += len(cfg[br])

    w_off = 2 * XB
    xn_off = w_off + ns * 256
    cols = xn_off + 2 * W * B

    in_maps = []
    for k in range(N_CORES):
        sl = slice(k * C_CORE, (k + 1) * C_CORE)
        xwk = np.zeros((PAIRS, 128, cols), np.uint8)
        planes = np.stack([x8[sl], r8[sl]], axis=2)  # [32, H, 2, WP, B]
        planes = planes.reshape(PAIRS, 2 * H, 2 * WP * B)
        xwk[:, :, :w_off] = planes.view(np.uint8)
        xwk[:, :, xn_off:] = xn[sl].reshape(PAIRS, 2 * H, W * B).view(np.uint8)
        # block-diag [K=128, S, 2, M=128]
        wmk = np.zeros((PAIRS, 128, ns, 2, 128), F8)
        Tk = slotT[sl].reshape(PAIRS, 2, ns, 2, H, H)
        for ci in range(2):
            blk = slice(ci * H, (ci + 1) * H)
            # [pairs, hi, s, slot, ho]
            wmk[:, blk, :, :, blk] = np.transpose(Tk[:, ci], (0, 3, 1, 2, 4))
        wmk = wmk[:, :, :, :, ::-1]
        wmk = np.swapaxes(wmk, 3, 4)  # -> [.., M, 2] interleave
        xwk[:, :, w_off:xn_off] = wmk.reshape(PAIRS, 128, ns * 256).view(np.uint8)
        in_maps.append({"xw": np.ascontiguousarray(xwk), "scbi": scbi[k]})
    return in_maps


def _assemble(results):
    y = np.empty((B, C, H, W), np.float32)
    for k in range(N_CORES):
        ytk = np.asarray(results[k]["yt"]).astype(np.float32)
        ytk = ytk.reshape(PAIRS, 2, H, W, B).transpose(4, 0, 1, 2, 3)
        y[:, k * C_CORE : (k + 1) * C_CORE] = ytk.reshape(B, C_CORE, H, W)
    return y


def kernel_run(inputs, trace=False, tmpdir=None):
    weights = [np.asarray(inputs[k], np.float32) for k in ("w5", "w7", "w3a", "w3b", "w3c")]
    if "cfg" not in _CACHE:
        _CACHE["cfg"] = _stream_cfg(weights)
        _CACHE["nc"] = build_nc(_CACHE["cfg"])
    cfg, nc = _CACHE["cfg"], _CACHE["nc"]
    in_maps = _host_prep(cfg, **inputs)
    res = run_bass_kernel_spmd(
        nc, in_maps, list(range(N_CORES)), trace=trace, tmpdir=tmpdir
    )
    return _assemble(res.results), res


_CACHE: dict = {}


def kernel(**inputs):
    out, _ = kernel_run(inputs, trace=False)
    return out
